# revision 8
# baseline (speedup 1.0000x reference)
"""Trainium2 Bass kernel for nn_F1_67379446940315 (histogram_binning F1 metric).

Computes: pred = argmax(y_pred, axis=1); conf = scatter-add confusion matrix;
then the (quirky, faithful-to-reference) per-class F1 reduction to a scalar.

Strategy (8 NeuronCores, data-parallel over N):
  - each core streams its shard of y_pred [131072, 128] f32 in tiles of
    [128 partitions, RPP rows x 128 classes]
  - per-row max via DVE segmented tensor_reduce (3D AP, axis=X)
  - pred one-hot mask S = is_equal(x, rowmax) via DVE tensor_scalar (per
    r-slice, per-partition scalar) -> bf16
  - true one-hot T = is_equal(iota, y_true) via DVE tensor_scalar -> bf16
  - PE matmul accumulation: conf_psum += T_r^T @ S_r  (contraction over the
    128 rows living on partitions), 8 matmuls per tile, all accumulating
    into one PSUM bank
  - per-core partial conf [128,128] f32 DMA'd out; host sums the 8 partials
    and does the tiny F1 reduction (negligible work, replicated per hint).
"""

import numpy as np
import ml_dtypes
from contextlib import ExitStack

import concourse.bass as bass
import concourse.bacc as bacc
import concourse.tile as tile
from concourse import mybir
from concourse import bass_utils

N_TOTAL = 1048576
C = 128
N_CORES = 8
SHARD = N_TOTAL // N_CORES  # 131072
EPS = np.float32(1e-12)

BF16 = mybir.dt.bfloat16
F32 = mybir.dt.float32


def build_conf_kernel(ctx, tc, conf_out, yp, yt, iota_ap, n_rows, rpp=8, reps=1):
    """Emit the per-core confusion-matrix kernel.

    conf_out: DRAM [128,128] f32 output AP
    yp:       DRAM [n_rows, 128] f32 input AP
    yt:       DRAM [128, n_rows//128] f32 input AP (laid out on host so that
              column c*rpp+r on partition p holds y_true[c*128*rpp + p*rpp + r])
    iota_ap:  DRAM [128, 128] bf16, each partition = 0..127
    """
    nc = tc.nc
    chunk_rows = 128 * rpp
    n_chunks = n_rows // chunk_rows
    assert n_rows % chunk_rows == 0
    fd = rpp * C  # free dim of an x tile

    # chunk view: [n_chunks, 128p, rpp*C]
    yp_v = yp.rearrange("(c p r) k -> c p (r k)", p=128, r=rpp)

    const_pool = ctx.enter_context(tc.tile_pool(name="const", bufs=1))
    x_pool = ctx.enter_context(tc.tile_pool(name="x", bufs=3))
    m_pool = ctx.enter_context(tc.tile_pool(name="m", bufs=3))
    s_pool = ctx.enter_context(tc.tile_pool(name="s", bufs=3))
    t_pool = ctx.enter_context(tc.tile_pool(name="t", bufs=3))
    psum_pool = ctx.enter_context(tc.tile_pool(name="psum", bufs=1, space="PSUM"))
    out_pool = ctx.enter_context(tc.tile_pool(name="out", bufs=1))

    iota_sb = const_pool.tile([128, C], BF16, tag="iota")
    nc.sync.dma_start(iota_sb[:], iota_ap)
    yt_sb = const_pool.tile([128, n_rows // 128], F32, tag="yt")
    nc.sync.dma_start(yt_sb[:], yt)

    conf_psum = psum_pool.tile([128, C], F32)

    def body():
        for c in range(n_chunks):
            x = x_pool.tile([128, fd], F32, tag="x")
            nc.sync.dma_start(x[:], yp_v[c])

            x3 = x[:].rearrange("p (r k) -> p r k", k=C)
            mx = m_pool.tile([128, rpp], F32, tag="mx")
            nc.vector.tensor_reduce(mx[:], x3, axis=mybir.AxisListType.X,
                                    op=mybir.AluOpType.max)

            s_t = s_pool.tile([128, fd], BF16, tag="s")
            t_t = t_pool.tile([128, fd], BF16, tag="t")
            for r in range(rpp):
                sl = slice(r * C, (r + 1) * C)
                nc.vector.tensor_scalar(
                    s_t[:, sl], x[:, sl], mx[:, r:r + 1], None,
                    mybir.AluOpType.is_equal)
                nc.vector.tensor_scalar(
                    t_t[:, sl], iota_sb[:], yt_sb[:, c * rpp + r:c * rpp + r + 1],
                    None, mybir.AluOpType.is_equal)
            for r in range(rpp):
                sl = slice(r * C, (r + 1) * C)
                nc.tensor.matmul(
                    conf_psum[:], t_t[:, sl], s_t[:, sl],
                    start=(c == 0 and r == 0),
                    stop=(c == n_chunks - 1 and r == rpp - 1))

    if reps == 1:
        body()
    else:
        with tc.For_i(0, reps, 1):
            body()

    conf_sb = out_pool.tile([128, C], F32)
    nc.scalar.copy(conf_sb[:], conf_psum[:])
    nc.sync.dma_start(conf_out, conf_sb[:])


def _host_layout_ytrue(yt_shard, rpp=8):
    """[SHARD] ints -> [128, SHARD//128] f32 in the kernel's expected layout."""
    n_chunks = yt_shard.shape[0] // (128 * rpp)
    return (yt_shard.reshape(n_chunks, 128, rpp)
            .transpose(1, 0, 2)
            .reshape(128, -1)
            .astype(np.float32))


def _iota_np():
    return np.tile(np.arange(C, dtype=ml_dtypes.bfloat16), (128, 1))


_compiled = {}


def _get_program(rpp=8, reps=1):
    key = (rpp, reps)
    if key in _compiled:
        return _compiled[key]
    nc = bacc.Bacc("TRN2", target_bir_lowering=False, debug=False)
    yp = nc.dram_tensor("yp", [SHARD, C], F32, kind="ExternalInput").ap()
    yt = nc.dram_tensor("yt", [128, SHARD // 128], F32, kind="ExternalInput").ap()
    iota_d = nc.dram_tensor("iota", [128, C], BF16, kind="ExternalInput").ap()
    conf = nc.dram_tensor("conf", [128, C], F32, kind="ExternalOutput").ap()
    with tile.TileContext(nc) as tc:
        with ExitStack() as ctx:
            build_conf_kernel(ctx, tc, conf, yp, yt, iota_d, SHARD, rpp=rpp,
                              reps=reps)
    nc.compile()
    _compiled[key] = nc
    return nc


def f1_from_conf(conf_f):
    """Replicates the reference's (quirky) F1 reduction on a [128,128] f32
    confusion matrix."""
    conf_f = conf_f.astype(np.float32)
    TP = np.diagonal(conf_f).astype(np.float32)
    FP = np.float32(C - 1) * conf_f[:, 1] + conf_f[:, 0]
    FN = np.float32(C - 1) * conf_f[1, :] + conf_f[0, :]
    sensitivity = TP / (TP + FN + EPS)
    precision = TP / (TP + FP + EPS)
    f1 = np.float32(2.0) * (precision * sensitivity / (precision + sensitivity + EPS))
    return np.array(np.mean(f1), dtype=np.float32)


def kernel(y_pred, y_true, _spmd_runner=None, **_ignored):
    y_pred = np.ascontiguousarray(np.asarray(y_pred), dtype=np.float32)
    y_true = np.asarray(y_true)
    assert y_pred.shape == (N_TOTAL, C)

    nc = _get_program()
    iota_np = _iota_np()
    in_maps = []
    yp_sh = y_pred.reshape(N_CORES, SHARD, C)
    yt_sh = y_true.reshape(N_CORES, SHARD)
    for i in range(N_CORES):
        in_maps.append({
            "yp": yp_sh[i],
            "yt": _host_layout_ytrue(yt_sh[i]),
            "iota": iota_np,
        })
    runner = _spmd_runner or bass_utils.run_bass_kernel_spmd
    res = runner(nc, in_maps, core_ids=list(range(N_CORES)))
    results = res.results if hasattr(res, "results") else res
    conf = np.zeros((128, C), dtype=np.float64)
    for r in results:
        conf += r["conf"].astype(np.float64)
    return f1_from_conf(conf.astype(np.float32))


# revision 31
# speedup vs baseline: 2.8564x; 2.8564x over previous
"""Trainium2 Bass kernel for nn_F1_67379446940315 (histogram_binning F1 metric).

Computes: pred = argmax(y_pred, axis=1); conf = scatter-add confusion matrix;
then the (quirky, faithful-to-reference) per-class F1 reduction to a scalar.

Strategy (8 NeuronCores, data-parallel over N):
  - each core streams its shard of y_pred [131072, 128] f32 in tiles of
    [128 partitions, RPP rows x 128 classes]
  - per-row max via DVE segmented tensor_reduce (3D AP, axis=X)
  - pred one-hot mask S = is_equal(x, rowmax) via DVE tensor_scalar (per
    r-slice, per-partition scalar) -> bf16
  - true one-hot T = is_equal(iota, y_true) via DVE tensor_scalar -> bf16
  - PE matmul accumulation: conf_psum += T_r^T @ S_r  (contraction over the
    128 rows living on partitions), 8 matmuls per tile, all accumulating
    into one PSUM bank
  - per-core partial conf [128,128] f32 DMA'd out; host sums the 8 partials
    and does the tiny F1 reduction (negligible work, replicated per hint).
"""

import numpy as np
import ml_dtypes
from contextlib import ExitStack

import concourse.bass as bass
import concourse.bacc as bacc
import concourse.tile as tile
from concourse import mybir
from concourse import bass_utils

N_TOTAL = 1048576
C = 128
N_CORES = 8
SHARD = N_TOTAL // N_CORES  # 131072
EPS = np.float32(1e-12)

BF16 = mybir.dt.bfloat16
F32 = mybir.dt.float32


def build_conf_kernel(ctx, tc, conf_out, yp, yt, iota_ap, n_rows, rpp=8, reps=1,
                      stages=("dma", "reduce", "mask", "onehot", "matmul"),
                      yt_dtype=F32, onehot_mode="ts", mask_mode="dve",
                      dma_split=1, act_frac=0.5, gps_frac=0.0, bufs_x=3, bufs_st=3):
    """Emit the per-core confusion-matrix kernel.

    conf_out: DRAM [128,256] f32 output AP (cols 0:128 = bank A is_equal
              counts; cols 128:256 = bank B Sign-inverted counts, host fixes)
    yp:       DRAM [n_rows, 128] f32 input AP
    yt:       DRAM [128, n_rows//128] f32 input AP (laid out on host so that
              column c*rpp+r on partition p holds y_true[c*128*rpp + p*rpp + r])
    iota_ap:  DRAM [128, 128] bf16, each partition = 0..127
    """
    nc = tc.nc
    chunk_rows = 128 * rpp
    n_chunks = n_rows // chunk_rows
    assert n_rows % chunk_rows == 0
    fd = rpp * C  # free dim of an x tile

    # chunk view: [n_chunks, 128p, rpp*C]
    yp_v = yp.rearrange("(c p r) k -> c p (r k)", p=128, r=rpp)

    const_pool = ctx.enter_context(tc.tile_pool(name="const", bufs=1))
    x_pool = ctx.enter_context(tc.tile_pool(name="x", bufs=bufs_x))
    m_pool = ctx.enter_context(tc.tile_pool(name="m", bufs=2 * bufs_st))
    s_pool = ctx.enter_context(tc.tile_pool(name="s", bufs=bufs_st))
    t_pool = ctx.enter_context(tc.tile_pool(name="t", bufs=bufs_st))
    psum_pool = ctx.enter_context(tc.tile_pool(name="psum", bufs=1, space="PSUM"))
    out_pool = ctx.enter_context(tc.tile_pool(name="out", bufs=1))

    iota_sb = const_pool.tile([128, C], BF16, tag="iota")
    nc.sync.dma_start(iota_sb[:], iota_ap)
    yt_sb = const_pool.tile([128, n_rows // 128], yt_dtype, tag="yt")
    nc.sync.dma_start(yt_sb[:], yt)
    trep_pool = ctx.enter_context(tc.tile_pool(name="trep", bufs=3))
    ones_sb = const_pool.tile([128, 16], BF16, tag="ones")
    nc.vector.memset(ones_sb[:], 1.0)

    conf_psum = psum_pool.tile([128, C], F32)
    confB_psum = psum_pool.tile([128, C], F32, tag="psumB")

    # chunk -> engine assignment for the mask stage ("mix" mode):
    # ACT handles act_frac of chunks via Sign (inverted mask, bank B + host
    # fix); GPSIMD handles gps_frac via whole-chunk tensor_tensor is_equal
    act_chunk = [False] * n_chunks
    gps_chunk = [False] * n_chunks
    if mask_mode == "mix":
        acc = gcc = 0.0
        for c in range(n_chunks):
            acc += act_frac
            if acc >= 1.0:
                acc -= 1.0
                act_chunk[c] = True
                continue
            gcc += gps_frac
            if gcc >= 1.0:
                gcc -= 1.0
                gps_chunk[c] = True
    a_list = [c for c in range(n_chunks) if not act_chunk[c]]
    b_list = [c for c in range(n_chunks) if act_chunk[c]]

    def body():
        for c in range(n_chunks):
            x = x_pool.tile([128, fd], F32, tag="x")
            if "dma" in stages:
                if dma_split == 1:
                    nc.sync.dma_start(x[:], yp_v[c])
                else:
                    h = fd // dma_split
                    engs = [nc.sync, nc.tensor, nc.scalar, nc.vector]
                    for k in range(dma_split):
                        engs[k % len(engs)].dma_start(
                            x[:, k * h:(k + 1) * h], yp_v[c][:, k * h:(k + 1) * h])

            x3 = x[:].rearrange("p (r k) -> p r k", k=C)
            mx = m_pool.tile([128, rpp], F32, tag="mx")
            if "reduce" in stages:
                nc.vector.tensor_reduce(mx[:], x3, axis=mybir.AxisListType.X,
                                        op=mybir.AluOpType.max)

            s_t = s_pool.tile([128, fd], BF16, tag="s")
            t_t = t_pool.tile([128, fd], BF16, tag="t")
            if "mask" in stages:
                if mask_mode == "mix" and act_chunk[c]:
                    # inverted mask on ACT: Sign(max - x) = 0 at argmax, 1 else
                    for r in range(rpp):
                        sl = slice(r * C, (r + 1) * C)
                        nc.scalar.activation(
                            s_t[:, sl], x[:, sl],
                            mybir.ActivationFunctionType.Sign,
                            bias=mx[:, r:r + 1], scale=-1.0)
                elif mask_mode in ("ttb", "mix"):
                    mx_b = mx[:].unsqueeze(2).broadcast_to([128, rpp, C])
                    eng = nc.gpsimd if gps_chunk[c] else nc.vector
                    eng.tensor_tensor(
                        s_t[:].rearrange("p (r k) -> p r k", k=C),
                        x3, mx_b, mybir.AluOpType.is_equal)
                else:
                    eng = nc.gpsimd if mask_mode == "gps" else nc.vector
                    for r in range(rpp):
                        sl = slice(r * C, (r + 1) * C)
                        eng.tensor_scalar(
                            s_t[:, sl], x[:, sl], mx[:, r:r + 1], None,
                            mybir.AluOpType.is_equal)
            if "onehot" in stages:
                if onehot_mode == "ts":
                    for r in range(rpp):
                        sl = slice(r * C, (r + 1) * C)
                        nc.vector.tensor_scalar(
                            t_t[:, sl], iota_sb[:],
                            yt_sb[:, c * rpp + r:c * rpp + r + 1],
                            None, mybir.AluOpType.is_equal)
                elif onehot_mode == "scatter":
                    # gpsimd local_scatter: per-partition one-hot build.
                    # yt holds host-precomputed int16 idx = (r%8)*128 + t.
                    half = 1024  # num_elems per call (must be < 2048)
                    rows_per_half = half // C  # 8
                    n_half = fd // half
                    for h in range(n_half):
                        nc.gpsimd.local_scatter(
                            t_t[:, h * half:(h + 1) * half],
                            ones_sb[:, :rows_per_half],
                            yt_sb[:, c * rpp + h * rows_per_half:
                                  c * rpp + (h + 1) * rows_per_half],
                            channels=128, num_elems=half,
                            num_idxs=rows_per_half)
                elif onehot_mode == "trep_tt":
                    # ACT materializes t replicated along the class dim;
                    # DVE compares against iota at bf16 2x
                    t_rep = trep_pool.tile([128, fd], BF16, tag="trep")
                    yt_bcast = (yt_sb[:, c * rpp:(c + 1) * rpp]
                                .unsqueeze(2).broadcast_to([128, rpp, C]))
                    nc.scalar.copy(t_rep[:].rearrange("p (r k) -> p r k", k=C),
                                   yt_bcast)
                    iota_b = (iota_sb[:].unsqueeze(1)
                              .broadcast_to([128, rpp, C]))
                    nc.vector.tensor_tensor(
                        t_t[:].rearrange("p (r k) -> p r k", k=C),
                        t_rep[:].rearrange("p (r k) -> p r k", k=C),
                        iota_b, mybir.AluOpType.is_equal)
                else:
                    raise ValueError(onehot_mode)
            if "matmul" in stages:
                if act_chunk[c]:
                    psum, first_c, last_c = confB_psum, b_list[0], b_list[-1]
                else:
                    psum, first_c, last_c = conf_psum, a_list[0], a_list[-1]
                for r in range(rpp):
                    sl = slice(r * C, (r + 1) * C)
                    nc.tensor.matmul(
                        psum[:], t_t[:, sl], s_t[:, sl],
                        start=(c == first_c and r == 0),
                        stop=(c == last_c and r == rpp - 1))

    if reps == 1:
        body()
    else:
        with tc.For_i(0, reps, 1):
            body()

    conf_sb = out_pool.tile([128, 2 * C], F32)
    if "matmul" in stages and a_list:
        nc.scalar.copy(conf_sb[:, :C], conf_psum[:])
    else:
        nc.vector.memset(conf_sb[:, :C], 0.0)
    if "matmul" in stages and b_list:
        nc.scalar.copy(conf_sb[:, C:], confB_psum[:])
    else:
        nc.vector.memset(conf_sb[:, C:], 0.0)
    nc.sync.dma_start(conf_out, conf_sb[:])


def _host_layout_ytrue(yt_shard, rpp=8, np_dtype=np.float32):
    """[SHARD] ints -> [128, SHARD//128] in the kernel's expected layout."""
    n_chunks = yt_shard.shape[0] // (128 * rpp)
    return (yt_shard.reshape(n_chunks, 128, rpp)
            .transpose(1, 0, 2)
            .reshape(128, -1)
            .astype(np_dtype))


def _host_layout_scatter_idx(yt_shard, rpp=8):
    """[SHARD] ints -> int16 [128, SHARD//128]: value (r%8)*128 + t in the
    kernel's (p, c*rpp+r) layout, for gpsimd local_scatter one-hot builds."""
    lay = _host_layout_ytrue(yt_shard, rpp, np.int64)
    ncols = lay.shape[1]
    offs = ((np.arange(ncols) % rpp) % 8) * C
    return (lay + offs[None, :]).astype(np.int16)


def _iota_np():
    return np.tile(np.arange(C, dtype=ml_dtypes.bfloat16), (128, 1))


_compiled = {}

# Best measured config on trn2 (see ablate.py): ~227us/iter vs ~196us DMA
# roofline per core.
BEST = dict(rpp=16, yt_dtype=mybir.dt.int16, onehot_mode="scatter",
            mask_mode="mix", act_frac=0.52, bufs_x=6, bufs_st=10)


def _get_program(rpp=8, reps=1,
                 stages=("dma", "reduce", "mask", "onehot", "matmul"),
                 yt_dtype=F32, onehot_mode="ts", mask_mode="dve", dma_split=1,
                 act_frac=0.5, gps_frac=0.0, bufs_x=3, bufs_st=3):
    key = (rpp, reps, tuple(stages), yt_dtype, onehot_mode, mask_mode, dma_split,
           act_frac, gps_frac, bufs_x, bufs_st)
    if key in _compiled:
        return _compiled[key]
    nc = bacc.Bacc("TRN2", target_bir_lowering=False, debug=False)
    yp = nc.dram_tensor("yp", [SHARD, C], F32, kind="ExternalInput").ap()
    yt = nc.dram_tensor("yt", [128, SHARD // 128], yt_dtype,
                        kind="ExternalInput").ap()
    iota_d = nc.dram_tensor("iota", [128, C], BF16, kind="ExternalInput").ap()
    conf = nc.dram_tensor("conf", [128, 2 * C], F32, kind="ExternalOutput").ap()
    with tile.TileContext(nc) as tc:
        with ExitStack() as ctx:
            build_conf_kernel(ctx, tc, conf, yp, yt, iota_d, SHARD, rpp=rpp,
                              reps=reps, stages=stages, yt_dtype=yt_dtype,
                              onehot_mode=onehot_mode, mask_mode=mask_mode,
                              dma_split=dma_split, act_frac=act_frac,
                              gps_frac=gps_frac, bufs_x=bufs_x, bufs_st=bufs_st)
    nc.compile()
    _compiled[key] = nc
    return nc


def conf_from_banks(res256):
    """[128,256] per-core result -> [128,128] f64 confusion counts."""
    res256 = res256.astype(np.float64)
    conf_a = res256[:, :C]
    m_b = res256[:, C:]
    cnt_b = m_b.sum(axis=1) / (C - 1)
    conf_b = cnt_b[:, None] - m_b
    return conf_a + conf_b


def f1_from_conf(conf_f):
    """Replicates the reference's (quirky) F1 reduction on a [128,128] f32
    confusion matrix."""
    conf_f = conf_f.astype(np.float32)
    TP = np.diagonal(conf_f).astype(np.float32)
    FP = np.float32(C - 1) * conf_f[:, 1] + conf_f[:, 0]
    FN = np.float32(C - 1) * conf_f[1, :] + conf_f[0, :]
    sensitivity = TP / (TP + FN + EPS)
    precision = TP / (TP + FP + EPS)
    f1 = np.float32(2.0) * (precision * sensitivity / (precision + sensitivity + EPS))
    return np.array(np.mean(f1), dtype=np.float32)


def make_in_maps(y_pred, y_true):
    """Shard + lay out the full inputs for the 8-core SPMD program."""
    iota_np = _iota_np()
    yp_sh = y_pred.reshape(N_CORES, SHARD, C)
    yt_sh = y_true.reshape(N_CORES, SHARD)
    rpp = BEST["rpp"]
    return [{
        "yp": yp_sh[i],
        "yt": _host_layout_scatter_idx(yt_sh[i], rpp),
        "iota": iota_np,
    } for i in range(N_CORES)]


def kernel(y_pred, y_true, _spmd_runner=None, **_ignored):
    y_pred = np.ascontiguousarray(np.asarray(y_pred), dtype=np.float32)
    y_true = np.asarray(y_true)
    assert y_pred.shape == (N_TOTAL, C)

    nc = _get_program(**BEST)
    in_maps = make_in_maps(y_pred, y_true)
    runner = _spmd_runner or bass_utils.run_bass_kernel_spmd
    res = runner(nc, in_maps, core_ids=list(range(N_CORES)))
    results = res.results if hasattr(res, "results") else res
    conf = np.zeros((128, C), dtype=np.float64)
    for r in results:
        conf += conf_from_banks(r["conf"])
    return f1_from_conf(conf.astype(np.float32))


# revision 34
# speedup vs baseline: 2.8806x; 1.0085x over previous
"""Trainium2 Bass kernel for nn_F1_67379446940315 (histogram_binning F1 metric).

Computes: pred = argmax(y_pred, axis=1); conf = scatter-add confusion matrix;
then the (quirky, faithful-to-reference) per-class F1 reduction to a scalar.

Strategy (8 NeuronCores, data-parallel over N; ~229us/iter vs ~196us DMA
roofline per core at ~334 GB/s):
  - each core streams its shard of y_pred [131072, 128] f32 in 1 MiB tiles
    [128 partitions, 16 rows x 128 classes]
  - per-row max via DVE segmented tensor_reduce (3D AP, axis=X)
  - pred one-hot mask, split across two engines to fit under the DMA floor:
      ~half the tiles on DVE: tensor_tensor is_equal(x, rowmax bcast) -> bf16
      ~half on ACT (ScalarE): Sign(rowmax - x) with per-partition bias = an
      INVERTED {0,1} mask; those tiles accumulate into a second PSUM bank
      and the host undoes the inversion exactly (integer algebra)
  - true one-hot T built by GPSIMD local_scatter from host-precomputed
    int16 indices (r%8)*128 + y_true -- zero DVE/ACT cost, hidden under DMA
  - PE matmul accumulation: conf_psum[bank] += T_r^T @ S_r (contraction over
    the 128 rows on partitions), 16 matmuls per tile, 1024 total per core
  - per-core [128, 256] (bank A | bank B) f32 DMA'd out; host reconstructs
    conf = A + (cntB - B), sums the 8 partials, and does the tiny F1
    reduction (negligible work, replicated per the sharding hint).
"""

import numpy as np
import ml_dtypes
from contextlib import ExitStack

import concourse.bass as bass
import concourse.bacc as bacc
import concourse.tile as tile
from concourse import mybir
from concourse import bass_utils

N_TOTAL = 1048576
C = 128
N_CORES = 8
SHARD = N_TOTAL // N_CORES  # 131072
EPS = np.float32(1e-12)

BF16 = mybir.dt.bfloat16
F32 = mybir.dt.float32


def build_conf_kernel(ctx, tc, conf_out, yp, yt, iota_ap, n_rows, rpp=8, reps=1,
                      stages=("dma", "reduce", "mask", "onehot", "matmul"),
                      yt_dtype=F32, onehot_mode="ts", mask_mode="dve",
                      dma_split=1, act_frac=0.5, gps_frac=0.0, bufs_x=3, bufs_st=3,
                      act_slices=8, loop_hints=False, dma_alt=False):
    """Emit the per-core confusion-matrix kernel.

    conf_out: DRAM [128,256] f32 output AP (cols 0:128 = bank A is_equal
              counts; cols 128:256 = bank B Sign-inverted counts, host fixes)
    yp:       DRAM [n_rows, 128] f32 input AP
    yt:       DRAM [128, n_rows//128] f32 input AP (laid out on host so that
              column c*rpp+r on partition p holds y_true[c*128*rpp + p*rpp + r])
    iota_ap:  DRAM [128, 128] bf16, each partition = 0..127
    """
    nc = tc.nc
    chunk_rows = 128 * rpp
    n_chunks = n_rows // chunk_rows
    assert n_rows % chunk_rows == 0
    fd = rpp * C  # free dim of an x tile

    # chunk view: [n_chunks, 128p, rpp*C]
    yp_v = yp.rearrange("(c p r) k -> c p (r k)", p=128, r=rpp)

    const_pool = ctx.enter_context(tc.tile_pool(name="const", bufs=1))
    x_pool = ctx.enter_context(tc.tile_pool(name="x", bufs=bufs_x))
    m_pool = ctx.enter_context(tc.tile_pool(name="m", bufs=2 * bufs_st))
    s_pool = ctx.enter_context(tc.tile_pool(name="s", bufs=bufs_st))
    t_pool = ctx.enter_context(tc.tile_pool(name="t", bufs=bufs_st))
    psum_pool = ctx.enter_context(tc.tile_pool(name="psum", bufs=1, space="PSUM"))
    out_pool = ctx.enter_context(tc.tile_pool(name="out", bufs=1))

    iota_sb = const_pool.tile([128, C], BF16, tag="iota")
    nc.sync.dma_start(iota_sb[:], iota_ap)
    yt_sb = const_pool.tile([128, n_rows // 128], yt_dtype, tag="yt")
    nc.sync.dma_start(yt_sb[:], yt)
    trep_pool = ctx.enter_context(tc.tile_pool(name="trep", bufs=3))
    ones_sb = const_pool.tile([128, 16], BF16, tag="ones")
    nc.vector.memset(ones_sb[:], 1.0)

    conf_psum = psum_pool.tile([128, C], F32)
    confB_psum = psum_pool.tile([128, C], F32, tag="psumB")

    # chunk -> engine assignment for the mask stage ("mix" mode):
    # ACT handles act_frac of chunks via Sign (inverted mask, bank B + host
    # fix); GPSIMD handles gps_frac via whole-chunk tensor_tensor is_equal
    act_chunk = [False] * n_chunks
    gps_chunk = [False] * n_chunks
    if mask_mode == "mix":
        acc = gcc = 0.0
        for c in range(n_chunks):
            acc += act_frac
            if acc >= 1.0:
                acc -= 1.0
                act_chunk[c] = True
                continue
            gcc += gps_frac
            if gcc >= 1.0:
                gcc -= 1.0
                gps_chunk[c] = True
    a_list = [c for c in range(n_chunks) if not act_chunk[c]]
    b_list = [c for c in range(n_chunks) if act_chunk[c]]
    if mask_mode == "rsplit":
        a_list = list(range(n_chunks)) if act_slices < rpp else []
        b_list = list(range(n_chunks)) if act_slices > 0 else []

    def body():
        for c in range(n_chunks):
            x = x_pool.tile([128, fd], F32, tag="x")
            if "dma" in stages:
                if dma_alt:
                    # two HWDGE rings: each engine loads the chunks whose
                    # mask it does NOT compute (sync ring for ACT chunks)
                    eng = nc.sync if (act_chunk[c] or c % 2 == 0) else nc.scalar
                    if mask_mode == "mix":
                        eng = nc.sync if act_chunk[c] else nc.scalar
                    eng.dma_start(x[:], yp_v[c])
                elif dma_split == 1:
                    nc.sync.dma_start(x[:], yp_v[c])
                else:
                    h = fd // dma_split
                    engs = [nc.sync, nc.tensor, nc.scalar, nc.vector]
                    for k in range(dma_split):
                        engs[k % len(engs)].dma_start(
                            x[:, k * h:(k + 1) * h], yp_v[c][:, k * h:(k + 1) * h])

            x3 = x[:].rearrange("p (r k) -> p r k", k=C)
            mx = m_pool.tile([128, rpp], F32, tag="mx")
            if "reduce" in stages:
                nc.vector.tensor_reduce(mx[:], x3, axis=mybir.AxisListType.X,
                                        op=mybir.AluOpType.max)

            s_t = s_pool.tile([128, fd], BF16, tag="s")
            t_t = t_pool.tile([128, fd], BF16, tag="t")
            if "mask" in stages:
                if mask_mode == "rsplit":
                    k = act_slices
                    for r in range(k):
                        sl = slice(r * C, (r + 1) * C)
                        nc.scalar.activation(
                            s_t[:, sl], x[:, sl],
                            mybir.ActivationFunctionType.Sign,
                            bias=mx[:, r:r + 1], scale=-1.0)
                    for r in range(k, rpp):
                        sl = slice(r * C, (r + 1) * C)
                        nc.vector.tensor_scalar(
                            s_t[:, sl], x[:, sl], mx[:, r:r + 1], None,
                            mybir.AluOpType.is_equal)
                elif mask_mode == "mix" and act_chunk[c]:
                    # inverted mask on ACT: Sign(max - x) = 0 at argmax, 1 else
                    for r in range(rpp):
                        sl = slice(r * C, (r + 1) * C)
                        nc.scalar.activation(
                            s_t[:, sl], x[:, sl],
                            mybir.ActivationFunctionType.Sign,
                            bias=mx[:, r:r + 1], scale=-1.0)
                elif mask_mode in ("ttb", "mix"):
                    mx_b = mx[:].unsqueeze(2).broadcast_to([128, rpp, C])
                    eng = nc.gpsimd if gps_chunk[c] else nc.vector
                    eng.tensor_tensor(
                        s_t[:].rearrange("p (r k) -> p r k", k=C),
                        x3, mx_b, mybir.AluOpType.is_equal)
                else:
                    eng = nc.gpsimd if mask_mode == "gps" else nc.vector
                    for r in range(rpp):
                        sl = slice(r * C, (r + 1) * C)
                        eng.tensor_scalar(
                            s_t[:, sl], x[:, sl], mx[:, r:r + 1], None,
                            mybir.AluOpType.is_equal)
            if "onehot" in stages:
                if onehot_mode == "ts":
                    for r in range(rpp):
                        sl = slice(r * C, (r + 1) * C)
                        nc.vector.tensor_scalar(
                            t_t[:, sl], iota_sb[:],
                            yt_sb[:, c * rpp + r:c * rpp + r + 1],
                            None, mybir.AluOpType.is_equal)
                elif onehot_mode == "scatter":
                    # gpsimd local_scatter: per-partition one-hot build.
                    # yt holds host-precomputed int16 idx = (r%8)*128 + t.
                    half = 1024  # num_elems per call (must be < 2048)
                    rows_per_half = half // C  # 8
                    n_half = fd // half
                    for h in range(n_half):
                        nc.gpsimd.local_scatter(
                            t_t[:, h * half:(h + 1) * half],
                            ones_sb[:, :rows_per_half],
                            yt_sb[:, c * rpp + h * rows_per_half:
                                  c * rpp + (h + 1) * rows_per_half],
                            channels=128, num_elems=half,
                            num_idxs=rows_per_half)
                elif onehot_mode == "trep_tt":
                    # ACT materializes t replicated along the class dim;
                    # DVE compares against iota at bf16 2x
                    t_rep = trep_pool.tile([128, fd], BF16, tag="trep")
                    yt_bcast = (yt_sb[:, c * rpp:(c + 1) * rpp]
                                .unsqueeze(2).broadcast_to([128, rpp, C]))
                    nc.scalar.copy(t_rep[:].rearrange("p (r k) -> p r k", k=C),
                                   yt_bcast)
                    iota_b = (iota_sb[:].unsqueeze(1)
                              .broadcast_to([128, rpp, C]))
                    nc.vector.tensor_tensor(
                        t_t[:].rearrange("p (r k) -> p r k", k=C),
                        t_rep[:].rearrange("p (r k) -> p r k", k=C),
                        iota_b, mybir.AluOpType.is_equal)
                else:
                    raise ValueError(onehot_mode)
            if "matmul" in stages:
                if mask_mode == "rsplit":
                    k = act_slices
                    for r in range(rpp):
                        sl = slice(r * C, (r + 1) * C)
                        if r < k:
                            nc.tensor.matmul(
                                confB_psum[:], t_t[:, sl], s_t[:, sl],
                                start=(c == 0 and r == 0),
                                stop=(c == n_chunks - 1 and r == k - 1))
                        else:
                            nc.tensor.matmul(
                                conf_psum[:], t_t[:, sl], s_t[:, sl],
                                start=(c == 0 and r == k),
                                stop=(c == n_chunks - 1 and r == rpp - 1))
                else:
                    if act_chunk[c]:
                        psum, first_c, last_c = confB_psum, b_list[0], b_list[-1]
                    else:
                        psum, first_c, last_c = conf_psum, a_list[0], a_list[-1]
                    for r in range(rpp):
                        sl = slice(r * C, (r + 1) * C)
                        nc.tensor.matmul(
                            psum[:], t_t[:, sl], s_t[:, sl],
                            start=(c == first_c and r == 0),
                            stop=(c == last_c and r == rpp - 1))

    if reps == 1:
        body()
    else:
        hints = (tuple(mybir.EngineType[e] for e in
                       ("DVE", "Activation", "PE", "SP", "Pool"))
                 if loop_hints else ())
        with tc.For_i(0, reps, 1, hint_engines=hints):
            body()

    conf_sb = out_pool.tile([128, 2 * C], F32)
    if "matmul" in stages and a_list:
        nc.scalar.copy(conf_sb[:, :C], conf_psum[:])
    else:
        nc.vector.memset(conf_sb[:, :C], 0.0)
    if "matmul" in stages and b_list:
        nc.scalar.copy(conf_sb[:, C:], confB_psum[:])
    else:
        nc.vector.memset(conf_sb[:, C:], 0.0)
    nc.sync.dma_start(conf_out, conf_sb[:])


def _host_layout_ytrue(yt_shard, rpp=8, np_dtype=np.float32):
    """[SHARD] ints -> [128, SHARD//128] in the kernel's expected layout."""
    n_chunks = yt_shard.shape[0] // (128 * rpp)
    return (yt_shard.reshape(n_chunks, 128, rpp)
            .transpose(1, 0, 2)
            .reshape(128, -1)
            .astype(np_dtype))


def _host_layout_scatter_idx(yt_shard, rpp=8):
    """[SHARD] ints -> int16 [128, SHARD//128]: value (r%8)*128 + t in the
    kernel's (p, c*rpp+r) layout, for gpsimd local_scatter one-hot builds."""
    lay = _host_layout_ytrue(yt_shard, rpp, np.int64)
    ncols = lay.shape[1]
    offs = ((np.arange(ncols) % rpp) % 8) * C
    return (lay + offs[None, :]).astype(np.int16)


def _iota_np():
    return np.tile(np.arange(C, dtype=ml_dtypes.bfloat16), (128, 1))


_compiled = {}

# Best measured config on trn2 (see ablate.py): ~227us/iter vs ~196us DMA
# roofline per core.
BEST = dict(rpp=16, yt_dtype=mybir.dt.int16, onehot_mode="scatter",
            mask_mode="mix", act_frac=0.52, bufs_x=6, bufs_st=10)


def _get_program(rpp=8, reps=1,
                 stages=("dma", "reduce", "mask", "onehot", "matmul"),
                 yt_dtype=F32, onehot_mode="ts", mask_mode="dve", dma_split=1,
                 act_frac=0.5, gps_frac=0.0, bufs_x=3, bufs_st=3,
                 act_slices=8, loop_hints=False, dma_alt=False):
    key = (rpp, reps, tuple(stages), yt_dtype, onehot_mode, mask_mode, dma_split,
           act_frac, gps_frac, bufs_x, bufs_st, act_slices, loop_hints, dma_alt)
    if key in _compiled:
        return _compiled[key]
    nc = bacc.Bacc("TRN2", target_bir_lowering=False, debug=False)
    yp = nc.dram_tensor("yp", [SHARD, C], F32, kind="ExternalInput").ap()
    yt = nc.dram_tensor("yt", [128, SHARD // 128], yt_dtype,
                        kind="ExternalInput").ap()
    iota_d = nc.dram_tensor("iota", [128, C], BF16, kind="ExternalInput").ap()
    conf = nc.dram_tensor("conf", [128, 2 * C], F32, kind="ExternalOutput").ap()
    with tile.TileContext(nc) as tc:
        with ExitStack() as ctx:
            build_conf_kernel(ctx, tc, conf, yp, yt, iota_d, SHARD, rpp=rpp,
                              reps=reps, stages=stages, yt_dtype=yt_dtype,
                              onehot_mode=onehot_mode, mask_mode=mask_mode,
                              dma_split=dma_split, act_frac=act_frac,
                              gps_frac=gps_frac, bufs_x=bufs_x, bufs_st=bufs_st,
                              act_slices=act_slices, loop_hints=loop_hints,
                              dma_alt=dma_alt)
    nc.compile()
    _compiled[key] = nc
    return nc


def conf_from_banks(res256):
    """[128,256] per-core result -> [128,128] f64 confusion counts."""
    res256 = res256.astype(np.float64)
    conf_a = res256[:, :C]
    m_b = res256[:, C:]
    cnt_b = m_b.sum(axis=1) / (C - 1)
    conf_b = cnt_b[:, None] - m_b
    return conf_a + conf_b


def f1_from_conf(conf_f):
    """Replicates the reference's (quirky) F1 reduction on a [128,128] f32
    confusion matrix."""
    conf_f = conf_f.astype(np.float32)
    TP = np.diagonal(conf_f).astype(np.float32)
    FP = np.float32(C - 1) * conf_f[:, 1] + conf_f[:, 0]
    FN = np.float32(C - 1) * conf_f[1, :] + conf_f[0, :]
    sensitivity = TP / (TP + FN + EPS)
    precision = TP / (TP + FP + EPS)
    f1 = np.float32(2.0) * (precision * sensitivity / (precision + sensitivity + EPS))
    return np.array(np.mean(f1), dtype=np.float32)


def make_in_maps(y_pred, y_true):
    """Shard + lay out the full inputs for the 8-core SPMD program."""
    iota_np = _iota_np()
    yp_sh = y_pred.reshape(N_CORES, SHARD, C)
    yt_sh = y_true.reshape(N_CORES, SHARD)
    rpp = BEST["rpp"]
    return [{
        "yp": yp_sh[i],
        "yt": _host_layout_scatter_idx(yt_sh[i], rpp),
        "iota": iota_np,
    } for i in range(N_CORES)]


def kernel(y_pred, y_true, _spmd_runner=None, **_ignored):
    y_pred = np.ascontiguousarray(np.asarray(y_pred), dtype=np.float32)
    y_true = np.asarray(y_true)
    assert y_pred.shape == (N_TOTAL, C)

    nc = _get_program(**BEST)
    in_maps = make_in_maps(y_pred, y_true)
    runner = _spmd_runner or bass_utils.run_bass_kernel_spmd
    res = runner(nc, in_maps, core_ids=list(range(N_CORES)))
    results = res.results if hasattr(res, "results") else res
    conf = np.zeros((128, C), dtype=np.float64)
    for r in results:
        conf += conf_from_banks(r["conf"])
    return f1_from_conf(conf.astype(np.float32))
